# revision 13
# baseline (speedup 1.0000x reference)
"""CNNMRF loss kernel for 8 trn2 NeuronCores.

Strategy
--------
The dominant work is two style-patch retrievals:
  resp = q @ sp_hat.T  (Q3=P3=3969, D3=2304 and Q4=P4=961, D4=4608)
followed by a row argmax. Only (max value, argmax index) per query is
needed on the host: the reconstruction loss is then reassembled exactly
in float64 from the original fp32 inputs, so device precision only
affects which near-tied style patch is selected.

Sharding: 2 query-groups x 4 style-patch-groups = 8 cores. Each core
holds its style chunk (pre-normalized, transposed, fp8-e4m3) resident
in SBUF and streams its query half through the PE with DoubleRow
matmuls (contraction 256/instruction). Per query tile, the row max m
comes from a DVE max-reduce over the fp32 PSUM responses; the argmax
index is extracted by computing 2^18*(resp - m) on the Scalar engine
and max-reducing (that + broadcast index map) on DVE: at the argmax the
shifted term is exactly 0, so the reduce returns the index.

Content and TV losses are O(MB) elementwise reductions, computed on host.
"""

import numpy as np
import ml_dtypes

import concourse.bacc as bacc
import concourse.mybir as mybir
import concourse.tile as tile
from concourse.bass_utils import run_bass_kernel_spmd

F32 = mybir.dt.float32
FP8 = mybir.dt.float8e4
X = mybir.AxisListType.X
ALU = mybir.AluOpType
ACT_ID = mybir.ActivationFunctionType.Identity
ACT_COPY = mybir.ActivationFunctionType.Copy
DR = mybir.MatmulPerfMode.DoubleRow
NPF8 = mybir.dt.np(mybir.dt.float8e4)

N_CORES = 8
N_QG = 2          # query groups
N_PG = 4          # style-patch groups
SCALE = 262144.0  # 2^18 argmax-extraction shift

# loss3: feat3 [256,128,128], patches 3x3 stride 2 -> Ho=63
C3, H3, D3, HO3 = 256, 128, 2304, 63
Q3 = HO3 * HO3            # 3969
KK3 = D3 // 256           # 9 double-row chunks
QH3 = 2048                # padded per-core query count (half of 3969 -> 1985)
NT3 = QH3 // 128          # 16 query tiles
NST3 = 4                  # supertiles of 512 queries
PH3 = 1024                # padded per-core style chunk (quarter of 3969 -> 993)

# loss4: feat4 [512,64,64] -> Ho=31
C4, H4, D4, HO4 = 512, 64, 4608, 31
Q4 = HO4 * HO4            # 961
KK4 = D4 // 256           # 18
QH4 = 512                 # padded per-core query count (481)
NT4 = QH4 // 128          # 4 query tiles
PH4 = 256                 # padded per-core style chunk (241)

CONTENT_WEIGHT = 1.0
TV_WEIGHT = 0.001

_NC = None  # cached compiled program


def _build_nc():
    nc = bacc.Bacc("TRN2", target_bir_lowering=False, debug=False,
                   enable_asserts=False, num_devices=N_CORES)

    s3_d = nc.dram_tensor("s3", [KK3, 128, 2, PH3], FP8, kind="ExternalInput")
    q3_d = nc.dram_tensor("q3", [KK3, 128, 2, QH3], FP8, kind="ExternalInput")
    i3_d = nc.dram_tensor("i3", [128, PH3], F32, kind="ExternalInput")
    s4_d = nc.dram_tensor("s4", [KK4, 128, 2, PH4], FP8, kind="ExternalInput")
    q4_d = nc.dram_tensor("q4", [KK4, 128, 2, QH4], FP8, kind="ExternalInput")
    i4_d = nc.dram_tensor("i4", [128, PH4], F32, kind="ExternalInput")

    out3m_d = nc.dram_tensor("out3m", [128, NT3], F32, kind="ExternalOutput")
    out3i_d = nc.dram_tensor("out3i", [128, NT3], F32, kind="ExternalOutput")
    out4m_d = nc.dram_tensor("out4m", [128, NT4], F32, kind="ExternalOutput")
    out4i_d = nc.dram_tensor("out4i", [128, NT4], F32, kind="ExternalOutput")

    with tile.TileContext(nc) as tc:
        with (
            tc.tile_pool(name="const", bufs=1) as cp,
            tc.tile_pool(name="q3s", bufs=2 * KK3) as qp,
            tc.tile_pool(name="psum", bufs=4, space="PSUM") as pp,
            tc.tile_pool(name="dtile", bufs=3) as dp,
            tc.tile_pool(name="sel", bufs=3) as selp,
            tc.tile_pool(name="neg", bufs=4) as negp,
            tc.tile_pool(name="outs", bufs=1) as op,
        ):
            # ---- resident constants; s3/q3-supertile-0 interleaved by k so
            # the warmup loop below computes while the stream lands ----
            s3_t, qts0 = [], []
            for k in range(KK3):
                t = cp.tile([128, 2, PH3], FP8, tag=f"s3_{k}")
                nc.scalar.dma_start(t[:], s3_d.ap()[k, :, :, :])
                s3_t.append(t)
                t = qp.tile([128, 2, 512], FP8, tag="q3s")
                nc.sync.dma_start(t[:], q3_d.ap()[k, :, :, 0:512])
                qts0.append(t)
                if k == 1:
                    i3_t = cp.tile([128, PH3], F32, tag="i3")
                    nc.scalar.dma_start(i3_t[:], i3_d.ap()[:, :])

            out3m = op.tile([128, NT3], F32, tag="out3m")
            out3i = op.tile([128, NT3], F32, tag="out3i")
            out4m = op.tile([128, NT4], F32, tag="out4m")
            out4i = op.tile([128, NT4], F32, tag="out4i")

            post_ctr = [0]

            def post(resp, mcol, icol, i_t, ph):
                # m = rowmax(resp); idx = rowmax(2^18*(resp-m) + (idx+1)map)
                nc.vector.reduce_max(mcol, resp[:], axis=X)
                negm = negp.tile([128, 1], F32, tag="negm")
                nc.scalar.mul(negm[:], mcol, -SCALE)
                d = dp.tile([128, ph], F32, tag="d", name=f"d_{ph}")
                nc.scalar.activation(d[:], resp[:], ACT_ID, bias=negm[:],
                                     scale=SCALE)
                sel = selp.tile([128, ph], F32, tag="sel", name=f"sel_{ph}")
                # alternate engines so neither gates the drain chain
                eng = nc.gpsimd if post_ctr[0] % 2 == 0 else nc.vector
                post_ctr[0] += 1
                eng.tensor_add(sel[:], d[:], i_t[:])
                nc.vector.reduce_max(icol, sel[:], axis=X)

            def mm3(resp, qt, tt, k):
                lhsT = qt[:, :, tt * 128:(tt + 1) * 128]
                nc.tensor.matmul(resp[:, 0:512], lhsT, s3_t[k][:, :, 0:512],
                                 start=(k == 0), stop=(k == KK3 - 1),
                                 perf_mode=DR)
                nc.tensor.matmul(resp[:, 512:1024], lhsT, s3_t[k][:, :, 512:1024],
                                 start=(k == 0), stop=(k == KK3 - 1),
                                 perf_mode=DR)

            # ---- supertile 0: pairwise k-outer (paces PE with the DMA
            # stream during the cold start) ----
            for pair in range(2):
                resps0 = [pp.tile([128, PH3], F32, tag="resp", name=f"r0_{pair}_{i}")
                          for i in range(2)]
                for k in range(KK3):
                    for i in range(2):
                        mm3(resps0[i], qts0[k], 2 * pair + i, k)
                for i in range(2):
                    t_idx = 2 * pair + i
                    post(resps0[i], out3m[:, t_idx:t_idx + 1],
                         out3i[:, t_idx:t_idx + 1], i3_t, PH3)

            s4_t, q4_t = [], []

            def loss4_tile(t_idx):
                resp = pp.tile([128, PH4], F32, tag="resp", name=f"r4_{t_idx}")
                for k in range(KK4):
                    nc.tensor.matmul(resp[:], q4_t[k][:, :, t_idx * 128:(t_idx + 1) * 128],
                                     s4_t[k][:, :, :], start=(k == 0),
                                     stop=(k == KK4 - 1), perf_mode=DR)
                post(resp, out4m[:, t_idx:t_idx + 1],
                     out4i[:, t_idx:t_idx + 1], i4_t, PH4)

            # ---- supertiles 1-3: tile-sequential; loss4 tiles interleaved
            # into supertiles 2-3 so the kernel tail ends on loss3 ----
            for st in range(1, NST3):
                qts = []
                for k in range(KK3):
                    t = qp.tile([128, 2, 512], FP8, tag="q3s")
                    nc.sync.dma_start(t[:], q3_d.ap()[k, :, :, st * 512:(st + 1) * 512])
                    qts.append(t)
                if st == 2:
                    # loss4 constant loads: behind st2's q3 stream, well
                    # before the first interleaved loss4 tile runs
                    i4_t = cp.tile([128, PH4], F32, tag="i4")
                    nc.scalar.dma_start(i4_t[:], i4_d.ap()[:, :])
                    for k in range(KK4):
                        t = cp.tile([128, 2, PH4], FP8, tag=f"s4_{k}")
                        nc.scalar.dma_start(t[:], s4_d.ap()[k, :, :, :])
                        s4_t.append(t)
                    for k in range(KK4):
                        t = cp.tile([128, 2, QH4], FP8, tag=f"q4_{k}")
                        nc.scalar.dma_start(t[:], q4_d.ap()[k, :, :, :])
                        q4_t.append(t)
                for tt in range(4):
                    t_idx = st * 4 + tt
                    resp = pp.tile([128, PH3], F32, tag="resp")
                    for k in range(KK3):
                        mm3(resp, qts[k], tt, k)
                    post(resp, out3m[:, t_idx:t_idx + 1],
                         out3i[:, t_idx:t_idx + 1], i3_t, PH3)
                    # spread loss4 tiles so the kernel tail ends on loss3
                    if st == 2 and tt == 3:
                        loss4_tile(0)
                    elif st == 3 and tt < 3:
                        loss4_tile(tt + 1)

            nc.sync.dma_start(out3m_d.ap()[:, :], out3m[:])
            nc.sync.dma_start(out3i_d.ap()[:, :], out3i[:])
            nc.sync.dma_start(out4m_d.ap()[:, :], out4m[:])
            nc.sync.dma_start(out4i_d.ap()[:, :], out4i[:])

    nc.compile()
    return nc


def _im2col(feat):
    """feat [C,H,W] f32 -> [Q, C*9] rows in (i,j) order, cols in (c,kh,kw) order."""
    sw = np.lib.stride_tricks.sliding_window_view(feat, (3, 3), axis=(1, 2))
    sw = sw[:, ::2, ::2]                       # [C, Ho, Wo, 3, 3]
    ho, wo = sw.shape[1], sw.shape[2]
    return np.ascontiguousarray(
        sw.transpose(1, 2, 0, 3, 4).reshape(ho * wo, feat.shape[0] * 9))


def _to_dr(buf):
    """[D, W] -> DoubleRow layout [D//256, 128, 2, W]."""
    D, W = buf.shape
    return np.ascontiguousarray(
        buf.reshape(D // 256, 2, 128, W).transpose(0, 2, 1, 3))


def _prep_side(q, sp_flat, QH, PH):
    """Build per-group device arrays for one loss.

    q: [Q, D] f32 query patches; sp_flat: [P, D] f32 style patches.
    """
    Qn, D = q.shape
    Pn = sp_flat.shape[0]
    n2 = (sp_flat.astype(np.float64) ** 2).sum(axis=1)
    inv = (1.0 / np.sqrt(n2)).astype(np.float32)
    shat = (sp_flat * inv[:, None]).astype(NPF8)   # [P, D] normalized, fp8

    qsplits = np.array_split(np.arange(Qn), N_QG)
    psplits = np.array_split(np.arange(Pn), N_PG)

    q_f8 = q.astype(NPF8)
    q_dev = []
    for qs in qsplits:
        buf = np.zeros((D, QH), dtype=NPF8)
        buf[:, :len(qs)] = q_f8[qs].T
        q_dev.append(_to_dr(buf))
    s_dev, i_dev = [], []
    for ps in psplits:
        buf = np.zeros((D, PH), dtype=NPF8)
        buf[:, :len(ps)] = shat[ps].T
        s_dev.append(_to_dr(buf))
        irow = np.zeros(PH, dtype=np.float32)
        irow[:len(ps)] = (ps + 1).astype(np.float32)   # global index + 1
        i_dev.append(np.broadcast_to(irow, (128, PH)).copy())
    return q_dev, s_dev, i_dev, qsplits, psplits


def _combine(res, key_m, key_i, qsplits):
    """Pick the winning style group per query, return global idx per query."""
    Qn = sum(len(qs) for qs in qsplits)
    idx = np.empty(Qn, dtype=np.int64)
    for qg, qs in enumerate(qsplits):
        cores = [qg * N_PG + pg for pg in range(N_PG)]
        m = np.stack([res[c][key_m].T.reshape(-1) for c in cores])   # [4, QH]
        ip = np.stack([res[c][key_i].T.reshape(-1) for c in cores])  # [4, QH]
        best = np.argmax(m, axis=0)
        chosen = ip[best, np.arange(ip.shape[1])][:len(qs)]
        assert chosen.min() >= 1.0
        idx[qs] = chosen.astype(np.int64) - 1
    return idx


def _mrf_loss_from_idx(q, sp_flat, idx):
    g = sp_flat[idx]
    q2 = np.einsum("qd,qd->q", q, q, dtype=np.float64)
    c = np.einsum("qd,qd->q", q, g, dtype=np.float64)
    n2 = np.einsum("qd,qd->q", g, g, dtype=np.float64)
    return float(np.mean(q2 - 2.0 * c + n2) / q.shape[1])


def kernel(synthesis, feat3, feat4, feat42, style_patches3, style_patches4,
           content_fm):
    global _NC
    synthesis = np.asarray(synthesis, dtype=np.float32)
    feat3 = np.asarray(feat3, dtype=np.float32)
    feat4 = np.asarray(feat4, dtype=np.float32)
    feat42 = np.asarray(feat42, dtype=np.float32)
    sp3 = np.asarray(style_patches3, dtype=np.float32).reshape(Q3, D3)
    sp4 = np.asarray(style_patches4, dtype=np.float32).reshape(Q4, D4)
    content_fm = np.asarray(content_fm, dtype=np.float32)

    q3 = _im2col(feat3[0])
    q4 = _im2col(feat4[0])

    q3_dev, s3_dev, i3_dev, qsp3, _ = _prep_side(q3, sp3, QH3, PH3)
    q4_dev, s4_dev, i4_dev, qsp4, _ = _prep_side(q4, sp4, QH4, PH4)

    in_maps = []
    for c in range(N_CORES):
        qg, pg = c // N_PG, c % N_PG
        in_maps.append({
            "s3": s3_dev[pg], "q3": q3_dev[qg], "i3": i3_dev[pg],
            "s4": s4_dev[pg], "q4": q4_dev[qg], "i4": i4_dev[pg],
        })

    if _NC is None:
        _NC = _build_nc()
    res = run_bass_kernel_spmd(_NC, in_maps, core_ids=list(range(N_CORES))).results

    idx3 = _combine(res, "out3m", "out3i", qsp3)
    idx4 = _combine(res, "out4m", "out4i", qsp4)
    mrf = _mrf_loss_from_idx(q3, sp3, idx3) + _mrf_loss_from_idx(q4, sp4, idx4)

    content = float(np.mean((feat42.astype(np.float64)
                             - content_fm.astype(np.float64)) ** 2))

    img = synthesis[0].transpose(1, 2, 0).astype(np.float64)
    scale = np.array([1.0 / 0.229, 1.0 / 0.224, 1.0 / 0.225])
    shift = np.array([0.485, 0.456, 0.406])
    t = img * scale + shift
    gx = np.concatenate([t[1:], t[-1:]], axis=0) - t
    gy = np.concatenate([t[:, 1:], t[:, -1:]], axis=1) - t
    tv = float((gx ** 2).mean() + (gy ** 2).mean())

    total = mrf + CONTENT_WEIGHT * content + TV_WEIGHT * tv
    return np.float32(total)


# revision 14
# speedup vs baseline: 1.0383x; 1.0383x over previous
"""CNNMRF loss kernel for 8 trn2 NeuronCores.

Strategy
--------
The dominant work is two style-patch retrievals:
  resp = q @ sp_hat.T  (Q3=P3=3969, D3=2304 and Q4=P4=961, D4=4608)
followed by a row argmax. Only (max value, argmax index) per query is
needed on the host: the reconstruction loss is then reassembled exactly
in float64 from the original fp32 inputs, so device precision only
affects which near-tied style patch is selected.

Sharding: 2 query-groups x 4 style-patch-groups = 8 cores. Each core
holds its style chunk (pre-normalized, transposed, fp8-e4m3) resident
in SBUF and streams its query half through the PE with DoubleRow
matmuls (contraction 256/instruction). Per query tile, the row max m
comes from a DVE max-reduce over the fp32 PSUM responses; the argmax
index is extracted by computing 2^18*(resp - m) on the Scalar engine
and max-reducing (that + broadcast index map) on DVE: at the argmax the
shifted term is exactly 0, so the reduce returns the index.

Content and TV losses are O(MB) elementwise reductions, computed on host.
"""

import numpy as np
import ml_dtypes

import concourse.bacc as bacc
import concourse.mybir as mybir
import concourse.tile as tile
from concourse.bass_utils import run_bass_kernel_spmd

F32 = mybir.dt.float32
FP8 = mybir.dt.float8e4
X = mybir.AxisListType.X
ALU = mybir.AluOpType
ACT_ID = mybir.ActivationFunctionType.Identity
ACT_COPY = mybir.ActivationFunctionType.Copy
DR = mybir.MatmulPerfMode.DoubleRow
NPF8 = mybir.dt.np(mybir.dt.float8e4)

N_CORES = 8
N_QG = 2          # query groups
N_PG = 4          # style-patch groups
SCALE = 262144.0  # 2^18 argmax-extraction shift

# loss3: feat3 [256,128,128], patches 3x3 stride 2 -> Ho=63
C3, H3, D3, HO3 = 256, 128, 2304, 63
Q3 = HO3 * HO3            # 3969
KK3 = D3 // 256           # 9 double-row chunks
QH3 = 2048                # padded per-core query count (half of 3969 -> 1985)
NT3 = QH3 // 128          # 16 query tiles
NST3 = 4                  # supertiles of 512 queries
PH3 = 1024                # padded per-core style chunk (quarter of 3969 -> 993)

# loss4: feat4 [512,64,64] -> Ho=31
C4, H4, D4, HO4 = 512, 64, 4608, 31
Q4 = HO4 * HO4            # 961
KK4 = D4 // 256           # 18
QH4 = 512                 # padded per-core query count (481)
NT4 = QH4 // 128          # 4 query tiles
PH4 = 256                 # padded per-core style chunk (241)

CONTENT_WEIGHT = 1.0
TV_WEIGHT = 0.001

_NC = None  # cached compiled program


def _build_nc():
    nc = bacc.Bacc("TRN2", target_bir_lowering=False, debug=False,
                   enable_asserts=False, num_devices=N_CORES)

    s3_d = nc.dram_tensor("s3", [KK3, 128, 2, PH3], FP8, kind="ExternalInput")
    q3_d = nc.dram_tensor("q3", [KK3, 128, 2, QH3], FP8, kind="ExternalInput")
    i3_d = nc.dram_tensor("i3", [128, PH3], F32, kind="ExternalInput")
    s4_d = nc.dram_tensor("s4", [KK4, 128, 2, PH4], FP8, kind="ExternalInput")
    q4_d = nc.dram_tensor("q4", [KK4, 128, 2, QH4], FP8, kind="ExternalInput")
    i4_d = nc.dram_tensor("i4", [128, PH4], F32, kind="ExternalInput")

    out3m_d = nc.dram_tensor("out3m", [128, NT3], F32, kind="ExternalOutput")
    out3i_d = nc.dram_tensor("out3i", [128, NT3], F32, kind="ExternalOutput")
    out4m_d = nc.dram_tensor("out4m", [128, NT4], F32, kind="ExternalOutput")
    out4i_d = nc.dram_tensor("out4i", [128, NT4], F32, kind="ExternalOutput")

    with tile.TileContext(nc) as tc:
        with (
            tc.tile_pool(name="const", bufs=1) as cp,
            tc.tile_pool(name="q3s", bufs=2 * KK3) as qp,
            tc.tile_pool(name="psum", bufs=4, space="PSUM") as pp,
            tc.tile_pool(name="dtile", bufs=3) as dp,
            tc.tile_pool(name="sel", bufs=3) as selp,
            tc.tile_pool(name="neg", bufs=4) as negp,
            tc.tile_pool(name="outs", bufs=1) as op,
        ):
            # ---- resident constants; s3/q3-supertile-0 interleaved by k so
            # the warmup loop below computes while the stream lands ----
            s3_t, qts0 = [], []
            for k in range(KK3):
                t = cp.tile([128, 2, PH3], FP8, tag=f"s3_{k}")
                nc.scalar.dma_start(t[:], s3_d.ap()[k, :, :, :])
                s3_t.append(t)
                t = qp.tile([128, 2, 512], FP8, tag="q3s")
                nc.sync.dma_start(t[:], q3_d.ap()[k, :, :, 0:512])
                qts0.append(t)
                if k == 1:
                    i3_t = cp.tile([128, PH3], F32, tag="i3")
                    nc.scalar.dma_start(i3_t[:], i3_d.ap()[:, :])

            out3m = op.tile([128, NT3], F32, tag="out3m")
            out3i = op.tile([128, NT3], F32, tag="out3i")
            out4m = op.tile([128, NT4], F32, tag="out4m")
            out4i = op.tile([128, NT4], F32, tag="out4i")

            post_ctr = [0]

            def post(resp, mcol, icol, i_t, ph):
                # m = rowmax(resp); idx = rowmax(2^18*(resp-m) + (idx+1)map)
                nc.vector.reduce_max(mcol, resp[:], axis=X)
                negm = negp.tile([128, 1], F32, tag="negm")
                nc.scalar.mul(negm[:], mcol, -SCALE)
                d = dp.tile([128, ph], F32, tag="d", name=f"d_{ph}")
                nc.scalar.activation(d[:], resp[:], ACT_ID, bias=negm[:],
                                     scale=SCALE)
                sel = selp.tile([128, ph], F32, tag="sel", name=f"sel_{ph}")
                # alternate engines so neither gates the drain chain
                eng = nc.gpsimd if post_ctr[0] % 2 == 0 else nc.vector
                post_ctr[0] += 1
                eng.tensor_add(sel[:], d[:], i_t[:])
                nc.vector.reduce_max(icol, sel[:], axis=X)

            def mm3(resp, qt, tt, k):
                lhsT = qt[:, :, tt * 128:(tt + 1) * 128]
                nc.tensor.matmul(resp[:, 0:512], lhsT, s3_t[k][:, :, 0:512],
                                 start=(k == 0), stop=(k == KK3 - 1),
                                 perf_mode=DR)
                nc.tensor.matmul(resp[:, 512:1024], lhsT, s3_t[k][:, :, 512:1024],
                                 start=(k == 0), stop=(k == KK3 - 1),
                                 perf_mode=DR)

            # ---- supertile 0: pairwise k-outer (paces PE with the DMA
            # stream during the cold start) ----
            for pair in range(2):
                resps0 = [pp.tile([128, PH3], F32, tag="resp", name=f"r0_{pair}_{i}")
                          for i in range(2)]
                for k in range(KK3):
                    for i in range(2):
                        mm3(resps0[i], qts0[k], 2 * pair + i, k)
                for i in range(2):
                    t_idx = 2 * pair + i
                    post(resps0[i], out3m[:, t_idx:t_idx + 1],
                         out3i[:, t_idx:t_idx + 1], i3_t, PH3)

            s4_t, q4_t = [], []

            def loss4_tile(t_idx):
                resp = pp.tile([128, PH4], F32, tag="resp", name=f"r4_{t_idx}")
                for k in range(KK4):
                    nc.tensor.matmul(resp[:], q4_t[k][:, :, t_idx * 128:(t_idx + 1) * 128],
                                     s4_t[k][:, :, :], start=(k == 0),
                                     stop=(k == KK4 - 1), perf_mode=DR)
                post(resp, out4m[:, t_idx:t_idx + 1],
                     out4i[:, t_idx:t_idx + 1], i4_t, PH4)

            # ---- supertiles 1-3: tile-sequential; loss4 tiles interleaved
            # into supertiles 2-3 so the kernel tail ends on loss3 ----
            for st in range(1, NST3):
                qts = []
                for k in range(KK3):
                    t = qp.tile([128, 2, 512], FP8, tag="q3s")
                    nc.sync.dma_start(t[:], q3_d.ap()[k, :, :, st * 512:(st + 1) * 512])
                    qts.append(t)
                if st == 2:
                    # loss4 constant loads: behind st2's q3 stream, well
                    # before the first interleaved loss4 tile runs
                    i4_t = cp.tile([128, PH4], F32, tag="i4")
                    nc.sync.dma_start(i4_t[:], i4_d.ap()[:, :])
                    for k in range(KK4):
                        t = cp.tile([128, 2, PH4], FP8, tag=f"s4_{k}")
                        nc.sync.dma_start(t[:], s4_d.ap()[k, :, :, :])
                        s4_t.append(t)
                    for k in range(KK4):
                        t = cp.tile([128, 2, QH4], FP8, tag=f"q4_{k}")
                        nc.sync.dma_start(t[:], q4_d.ap()[k, :, :, :])
                        q4_t.append(t)
                for tt in range(4):
                    t_idx = st * 4 + tt
                    resp = pp.tile([128, PH3], F32, tag="resp")
                    for k in range(KK3):
                        mm3(resp, qts[k], tt, k)
                    post(resp, out3m[:, t_idx:t_idx + 1],
                         out3i[:, t_idx:t_idx + 1], i3_t, PH3)
                    # spread loss4 tiles so the kernel tail ends on loss3
                    if st == 2 and tt == 3:
                        loss4_tile(0)
                    elif st == 3 and tt < 3:
                        loss4_tile(tt + 1)

            nc.sync.dma_start(out3m_d.ap()[:, :], out3m[:])
            nc.sync.dma_start(out3i_d.ap()[:, :], out3i[:])
            nc.sync.dma_start(out4m_d.ap()[:, :], out4m[:])
            nc.sync.dma_start(out4i_d.ap()[:, :], out4i[:])

    nc.compile()
    return nc


def _im2col(feat):
    """feat [C,H,W] f32 -> [Q, C*9] rows in (i,j) order, cols in (c,kh,kw) order."""
    sw = np.lib.stride_tricks.sliding_window_view(feat, (3, 3), axis=(1, 2))
    sw = sw[:, ::2, ::2]                       # [C, Ho, Wo, 3, 3]
    ho, wo = sw.shape[1], sw.shape[2]
    return np.ascontiguousarray(
        sw.transpose(1, 2, 0, 3, 4).reshape(ho * wo, feat.shape[0] * 9))


def _to_dr(buf):
    """[D, W] -> DoubleRow layout [D//256, 128, 2, W]."""
    D, W = buf.shape
    return np.ascontiguousarray(
        buf.reshape(D // 256, 2, 128, W).transpose(0, 2, 1, 3))


def _prep_side(q, sp_flat, QH, PH):
    """Build per-group device arrays for one loss.

    q: [Q, D] f32 query patches; sp_flat: [P, D] f32 style patches.
    """
    Qn, D = q.shape
    Pn = sp_flat.shape[0]
    n2 = (sp_flat.astype(np.float64) ** 2).sum(axis=1)
    inv = (1.0 / np.sqrt(n2)).astype(np.float32)
    shat = (sp_flat * inv[:, None]).astype(NPF8)   # [P, D] normalized, fp8

    qsplits = np.array_split(np.arange(Qn), N_QG)
    psplits = np.array_split(np.arange(Pn), N_PG)

    q_f8 = q.astype(NPF8)
    q_dev = []
    for qs in qsplits:
        buf = np.zeros((D, QH), dtype=NPF8)
        buf[:, :len(qs)] = q_f8[qs].T
        q_dev.append(_to_dr(buf))
    s_dev, i_dev = [], []
    for ps in psplits:
        buf = np.zeros((D, PH), dtype=NPF8)
        buf[:, :len(ps)] = shat[ps].T
        s_dev.append(_to_dr(buf))
        irow = np.zeros(PH, dtype=np.float32)
        irow[:len(ps)] = (ps + 1).astype(np.float32)   # global index + 1
        i_dev.append(np.broadcast_to(irow, (128, PH)).copy())
    return q_dev, s_dev, i_dev, qsplits, psplits


def _combine(res, key_m, key_i, qsplits):
    """Pick the winning style group per query, return global idx per query."""
    Qn = sum(len(qs) for qs in qsplits)
    idx = np.empty(Qn, dtype=np.int64)
    for qg, qs in enumerate(qsplits):
        cores = [qg * N_PG + pg for pg in range(N_PG)]
        m = np.stack([res[c][key_m].T.reshape(-1) for c in cores])   # [4, QH]
        ip = np.stack([res[c][key_i].T.reshape(-1) for c in cores])  # [4, QH]
        best = np.argmax(m, axis=0)
        chosen = ip[best, np.arange(ip.shape[1])][:len(qs)]
        assert chosen.min() >= 1.0
        idx[qs] = chosen.astype(np.int64) - 1
    return idx


def _mrf_loss_from_idx(q, sp_flat, idx):
    g = sp_flat[idx]
    q2 = np.einsum("qd,qd->q", q, q, dtype=np.float64)
    c = np.einsum("qd,qd->q", q, g, dtype=np.float64)
    n2 = np.einsum("qd,qd->q", g, g, dtype=np.float64)
    return float(np.mean(q2 - 2.0 * c + n2) / q.shape[1])


def kernel(synthesis, feat3, feat4, feat42, style_patches3, style_patches4,
           content_fm):
    global _NC
    synthesis = np.asarray(synthesis, dtype=np.float32)
    feat3 = np.asarray(feat3, dtype=np.float32)
    feat4 = np.asarray(feat4, dtype=np.float32)
    feat42 = np.asarray(feat42, dtype=np.float32)
    sp3 = np.asarray(style_patches3, dtype=np.float32).reshape(Q3, D3)
    sp4 = np.asarray(style_patches4, dtype=np.float32).reshape(Q4, D4)
    content_fm = np.asarray(content_fm, dtype=np.float32)

    q3 = _im2col(feat3[0])
    q4 = _im2col(feat4[0])

    q3_dev, s3_dev, i3_dev, qsp3, _ = _prep_side(q3, sp3, QH3, PH3)
    q4_dev, s4_dev, i4_dev, qsp4, _ = _prep_side(q4, sp4, QH4, PH4)

    in_maps = []
    for c in range(N_CORES):
        qg, pg = c // N_PG, c % N_PG
        in_maps.append({
            "s3": s3_dev[pg], "q3": q3_dev[qg], "i3": i3_dev[pg],
            "s4": s4_dev[pg], "q4": q4_dev[qg], "i4": i4_dev[pg],
        })

    if _NC is None:
        _NC = _build_nc()
    res = run_bass_kernel_spmd(_NC, in_maps, core_ids=list(range(N_CORES))).results

    idx3 = _combine(res, "out3m", "out3i", qsp3)
    idx4 = _combine(res, "out4m", "out4i", qsp4)
    mrf = _mrf_loss_from_idx(q3, sp3, idx3) + _mrf_loss_from_idx(q4, sp4, idx4)

    content = float(np.mean((feat42.astype(np.float64)
                             - content_fm.astype(np.float64)) ** 2))

    img = synthesis[0].transpose(1, 2, 0).astype(np.float64)
    scale = np.array([1.0 / 0.229, 1.0 / 0.224, 1.0 / 0.225])
    shift = np.array([0.485, 0.456, 0.406])
    t = img * scale + shift
    gx = np.concatenate([t[1:], t[-1:]], axis=0) - t
    gy = np.concatenate([t[:, 1:], t[:, -1:]], axis=1) - t
    tv = float((gx ** 2).mean() + (gy ** 2).mean())

    total = mrf + CONTENT_WEIGHT * content + TV_WEIGHT * tv
    return np.float32(total)
